# revision 11
# baseline (speedup 1.0000x reference)
"""Multi-headed attention (B=4, S=2048, D=512, H=8) on 8 TRN2 NeuronCores.

Sharding: core c handles batch b = c//2 and head-group hg = c%2 (4 of the 8
heads, i.e. a 256-wide slice of the model dim). Each core computes the full
attention for its (batch, 4 heads) and a partial output projection through
the matching 256-column slice of Wo. The host sums the two partials per
batch and adds the output bias.

Per-core kernel (all matmul operands are float32r = TF32-like):
  QT/KT [256, 2048] = W{q,k}T.T @ XT   (e on partitions, seq on free dim)
  V     [2048, 256] = XT.T @ WvT       (+ a ones column per head)
  per (head-pair, q-block 512, k-strip 128):
      scoresT [128, 1024] psum: two row-packed K=64 matmuls (heads at
          partition offsets 0 / 64)
      exp: one ACTIVATE over the [128, 1024] psum tile -> SBUF (f32r)
      attnV: per head, [65, 512] psum += V_aug[k,65].T @ expT[k,512]
          (row 64 = ones -> softmax denominator)
  divide: denom row -> SBUF, K=1 matmul broadcast to 64 partitions,
      fast reciprocal, DVE multiply -> attnT (normalized, f32r)
  y [2048, 512] partial: per s-tile, 4 accumulating K=64 matmuls
      (attnT_h.T @ WoT_h) -> DVE copy -> DMA out.

Phase-1 projections are emitted interleaved with the first attention
blocks so the scalar engine (exp) starts early; y-projection for block
qb is emitted during block qb+1 so its matmuls fill PE gaps.
"""

import numpy as np

S = 2048          # sequence length
D = 512           # model dim
EL = 256          # local (per-core) slice of model dim = 4 heads * 64
H = 4             # local heads
DH = 64           # head dim
P = 128           # partitions
NKC = D // P      # k chunks for projections (4)
NST = S // P      # s tiles of 128 (16)
NQB = S // 512    # q blocks of 512 (4)

_CACHE = {}


def _build_nc(pt_bufs=6, sc_bufs=2, av_bufs=2, yp_bufs=2):
    import concourse.bacc as bacc
    import concourse.mybir as mybir
    import concourse.tile as tile

    F32 = mybir.dt.float32
    F32R = mybir.dt.float32r
    EXP = mybir.ActivationFunctionType.Exp

    nc = bacc.Bacc()

    XT = nc.declare_dram_parameter("XT", [D, S], F32R, isOutput=False)
    WQT = nc.declare_dram_parameter("WQT", [D, EL], F32R, isOutput=False)
    WKT = nc.declare_dram_parameter("WKT", [D, EL], F32R, isOutput=False)
    WVT = nc.declare_dram_parameter("WVT", [D, EL], F32R, isOutput=False)
    WOT = nc.declare_dram_parameter("WOT", [EL, D], F32R, isOutput=False)
    BQ = nc.declare_dram_parameter("BQ", [EL], F32, isOutput=False)
    BK = nc.declare_dram_parameter("BK", [EL], F32, isOutput=False)
    BVB = nc.declare_dram_parameter("BVB", [P, EL], F32, isOutput=False)
    Y = nc.declare_dram_parameter("Y", [S, D], F32, isOutput=True)

    with tile.TileContext(nc) as tc:
        with (
            tc.tile_pool(name="persist", bufs=1) as pp,
            tc.tile_pool(name="pt", bufs=pt_bufs) as ptp,
            tc.tile_pool(name="rc", bufs=2) as rcp,
            tc.tile_pool(name="rb", bufs=2) as rbp,
            tc.tile_pool(name="ysb", bufs=3) as ysbp,
            tc.tile_pool(name="sc", bufs=sc_bufs, space="PSUM") as scp,
            tc.tile_pool(name="av", bufs=av_bufs, space="PSUM") as avp,
            tc.tile_pool(name="yp", bufs=yp_bufs, space="PSUM") as ypp,
        ):
            # ---- persistent SBUF tiles ----
            xt = pp.tile([P, NKC, S], F32R)          # X^T, d on partitions
            wqt = pp.tile([P, NKC, EL], F32R)
            wkt = pp.tile([P, NKC, EL], F32R)
            wvt = pp.tile([P, NKC, EL], F32R)
            wot = pp.tile([DH, H, D], F32R)          # d_local on partitions 0-63
            bq = pp.tile([P, 2], F32)
            bk = pp.tile([P, 2], F32)
            bvb = pp.tile([P, EL], F32)
            qt = pp.tile([P, 2, S], F32R)            # Q^T (e-chunk on outer)
            kt = pp.tile([P, 2, S], F32R)
            vv = pp.tile([P, NST, H, DH + 1], F32R)  # V + ones col per head
            at = pp.tile([DH, H, S], F32R)           # normalized attn^T

            # ---- loads ----
            nc.sync.dma_start(xt[:], XT.rearrange("(o p) s -> p o s", p=P))
            nc.sync.dma_start(wqt[:], WQT.rearrange("(o p) e -> p o e", p=P))
            nc.sync.dma_start(wkt[:], WKT.rearrange("(o p) e -> p o e", p=P))
            nc.sync.dma_start(wvt[:], WVT.rearrange("(o p) e -> p o e", p=P))
            nc.sync.dma_start(wot[:], WOT.rearrange("(h p) e -> p h e", p=DH))
            nc.sync.dma_start(bq[:], BQ.rearrange("(o p) -> p o", p=P))
            nc.sync.dma_start(bk[:], BK.rearrange("(o p) -> p o", p=P))
            nc.sync.dma_start(bvb[:], BVB[:])

            # fill V tile with 1.0 (broadcast copy; memset can't write f32r);
            # V-proj copies overwrite cols 0:DH of each head, leaving col DH
            # as the ones column that produces the softmax denominator
            ones = pp.tile([P, 1, 1, 1], F32)
            nc.vector.memset(ones[:], 1.0)
            nc.vector.tensor_copy(vv[:], ones[:].to_broadcast([P, NST, H, DH + 1]))
            # ones row at partition 64, lhsT of the K=1 broadcast matmul
            ones2 = pp.tile([DH + 1, DH], F32R)
            nc.vector.tensor_copy(
                ones2[:], ones[0 : DH + 1, 0, 0, :].to_broadcast([DH + 1, DH])
            )

            # ---- emission helpers ----
            def proj_qk(dst, w, b, e, blk):
                ps = scp.tile([P, 1024], F32, tag="sc")
                for sb in range(2):
                    s0 = (blk * 2 + sb) * 512
                    for k in range(NKC):
                        nc.tensor.matmul(
                            ps[:, sb * 512 : (sb + 1) * 512],
                            w[:, k, e * P : (e + 1) * P],
                            xt[:, k, s0 : s0 + 512],
                            start=(k == 0),
                            stop=(k == NKC - 1),
                            skip_group_check=True,
                        )
                nc.vector.tensor_scalar_add(
                    dst[:, e, blk * 1024 : (blk + 1) * 1024],
                    ps[:],
                    b[:, e : e + 1],
                )

            def proj_v():
                for st in range(NST):
                    ps = ypp.tile([P, 512], F32, tag="yp")
                    for k in range(NKC):
                        nc.tensor.matmul(
                            ps[:, :EL],
                            xt[:, k, st * P : (st + 1) * P],
                            wvt[:, k, :],
                            start=(k == 0),
                            stop=(k == NKC - 1),
                            skip_group_check=True,
                        )
                    nc.vector.tensor_add(
                        vv[:, st, :, 0:DH],
                        ps[:, :EL].rearrange("p (h d) -> p h d", h=H),
                        bvb[:].rearrange("p (h d) -> p h d", h=H),
                    )

            def strips(qb, p):
                """Score + exp + attnV accumulation for one (q-block, pair)."""
                q0 = qb * 512
                avA = avp.tile([DH + 1, 512], F32, tag="av")
                avB = avp.tile([DH + 1, 512], F32, tag="av")
                for ks in range(NST):
                    sc = scp.tile([P, 1024], F32, tag="sc")
                    k0 = ks * P
                    nc.tensor.matmul(
                        sc[:, 0:512],
                        kt[0:DH, p, k0 : k0 + P],
                        qt[0:DH, p, q0 : q0 + 512],
                    )
                    nc.tensor.matmul(
                        sc[:, 512:1024],
                        kt[DH:P, p, k0 : k0 + P],
                        qt[DH:P, p, q0 : q0 + 512],
                    )
                    pt = ptp.tile([P, 1024], F32R, tag="pt")
                    nc.scalar.activation(pt[:], sc[:], EXP)
                    for hp, av in ((0, avA), (1, avB)):
                        h = 2 * p + hp
                        nc.tensor.matmul(
                            av[:],
                            vv[:, ks, h, :],
                            pt[:, hp * 512 : (hp + 1) * 512],
                            start=(ks == 0),
                            stop=(ks == NST - 1),
                            skip_group_check=True,
                        )
                return avA, avB

            def trans(qb, p, avA, avB):
                """Normalize: denom -> bcast matmul -> fast recip -> mult."""
                q0 = qb * 512
                for hp, av in ((0, avA), (1, avB)):
                    h = 2 * p + hp
                    rc = rcp.tile([DH + 1, 512], F32R, tag="rc")
                    with nc.allow_low_precision(
                        reason="f32r recip row feeds broadcast matmul"
                    ):
                        nc.vector.reciprocal(
                            rc[DH : DH + 1, :], av[DH : DH + 1, :]
                        )
                    rb_ps = ypp.tile([DH, 512], F32, tag="yp")
                    nc.tensor.matmul(
                        rb_ps[:], ones2[DH : DH + 1, :], rc[DH : DH + 1, :]
                    )
                    rb = rbp.tile([DH, 512], F32, tag="rb")
                    nc.vector.tensor_copy(rb[:], rb_ps[:])
                    nc.vector.tensor_mul(
                        at[:, h, q0 : q0 + 512], av[0:DH, :], rb[:]
                    )

            def ytile(st):
                yp = ypp.tile([P, 512], F32, tag="yp")
                for h in range(H):
                    nc.tensor.matmul(
                        yp[:],
                        at[:, h, st * P : (st + 1) * P],
                        wot[:, h, :],
                        start=(h == 0),
                        stop=(h == H - 1),
                        skip_group_check=True,
                    )
                ysb = ysbp.tile([P, 512], F32, tag="ysb")
                nc.vector.tensor_copy(ysb[:], yp[:])
                nc.sync.dma_start(Y[st * P : (st + 1) * P, :], ysb[:])

            # ---- emission order: start attention ASAP, fill PE gaps with
            # the remaining projections and the (deferred) y-projection ----
            proj_qk(qt, wqt, bq, 0, 0)
            proj_qk(kt, wkt, bk, 0, 0)
            proj_v()

            a0, b0 = strips(0, 0)
            trans(0, 0, a0, b0)
            proj_qk(qt, wqt, bq, 1, 0)
            proj_qk(kt, wkt, bk, 1, 0)
            a0, b0 = strips(0, 1)
            trans(0, 1, a0, b0)
            proj_qk(qt, wqt, bq, 0, 1)
            proj_qk(kt, wkt, bk, 0, 1)
            proj_qk(qt, wqt, bq, 1, 1)
            proj_qk(kt, wkt, bk, 1, 1)

            for qb in range(1, NQB):
                for p in range(2):
                    a0, b0 = strips(qb, p)
                    trans(qb, p, a0, b0)
                    if p == 0:
                        # y for the previous q-block fills PE gaps here
                        for st in range((qb - 1) * 4, qb * 4):
                            ytile(st)
            for st in range((NQB - 1) * 4, NQB * 4):
                ytile(st)

    nc.finalize()
    return nc


def _get_nc():
    if "nc" not in _CACHE:
        _CACHE["nc"] = _build_nc()
    return _CACHE["nc"]


def _prep_inputs(X, Wq, bq, Wk, bk, Wv, bv, Wo, bo):
    f = lambda a: np.ascontiguousarray(np.asarray(a), dtype=np.float32)
    X, Wq, bq, Wk, bk, Wv, bv, Wo, bo = map(f, (X, Wq, bq, Wk, bk, Wv, bv, Wo, bo))
    B = X.shape[0]
    scale = np.float32(1.0 / np.sqrt(DH))
    XT = [np.ascontiguousarray(X[b].T) for b in range(B)]
    in_maps = []
    for c in range(2 * B):
        b, hg = divmod(c, 2)
        sl = slice(hg * EL, (hg + 1) * EL)
        in_maps.append(
            {
                "XT": XT[b],
                "WQT": np.ascontiguousarray((Wq[sl] * scale).T),
                "WKT": np.ascontiguousarray(Wk[sl].T),
                "WVT": np.ascontiguousarray(Wv[sl].T),
                "WOT": np.ascontiguousarray(Wo[:, sl].T),
                "BQ": np.ascontiguousarray(bq[sl] * scale),
                "BK": np.ascontiguousarray(bk[sl]),
                "BVB": np.ascontiguousarray(np.tile(bv[sl], (P, 1))),
            }
        )
    return in_maps, bo, B


def run(inputs, trace=False, trace_cores=None):
    """Run the kernel; returns (Y_full, exec_time_ns or None)."""
    from concourse.bass_utils import run_bass_kernel_spmd

    in_maps, bo, B = _prep_inputs(**inputs)
    nc = _get_nc()
    kw = {}
    if trace:
        kw = dict(trace=True, trace_cores=trace_cores or list(range(2 * B)))
    res = run_bass_kernel_spmd(nc, in_maps, list(range(2 * B)), **kw)
    Y = np.stack(
        [
            res.results[2 * b]["Y"] + res.results[2 * b + 1]["Y"] + bo
            for b in range(B)
        ]
    )
    return Y, getattr(res, "exec_time_ns", None)


def kernel(X, Wq, bq, Wk, bk, Wv, bv, Wo, bo):
    Y, _ = run(
        dict(X=X, Wq=Wq, bq=bq, Wk=Wk, bk=bk, Wv=Wv, bv=bv, Wo=Wo, bo=bo)
    )
    return Y


# revision 31
# speedup vs baseline: 1.3274x; 1.3274x over previous
"""Multi-headed attention (B=4, S=2048, D=512, H=8) on 8 TRN2 NeuronCores.

Sharding: core c handles batch b = c//2 and head-group hg = c%2 (4 of the 8
heads, i.e. a 256-wide slice of the model dim). Each core computes the full
attention for its (batch, 4 heads) and a partial output projection through
the matching 256-column slice of Wo. The host sums the two partials per
batch and adds the output bias.

Per-core kernel (all matmul operands are float32r = TF32-like):
  QT/KT [256, 2048] = W{q,k}T.T @ XT   (e on partitions, seq on free dim)
  V     [2048, 256] = XT.T @ WvT       (+ a ones column per head)
  per (head-pair, q-block 512, k-strip 128):
      scoresT [128, 1024] psum: two row-packed K=64 matmuls (heads at
          partition offsets 0 / 64)
      exp: one ACTIVATE over the [128, 1024] psum tile -> SBUF (f32r)
      attnV: per head, [65, 512] psum += V_aug[k,65].T @ expT[k,512]
          (row 64 = ones -> softmax denominator)
  divide: denom row -> SBUF, K=1 matmul broadcast to 64 partitions,
      fast reciprocal, DVE multiply -> attnT (normalized, f32r)
  y [2048, 512] partial: per s-tile, 4 accumulating K=64 matmuls
      (attnT_h.T @ WoT_h) -> DVE copy -> DMA out.

Phase-1 projections are emitted interleaved with the first attention
blocks so the scalar engine (exp) starts early; y-projection for block
qb is emitted during block qb+1 so its matmuls fill PE gaps.
"""

import numpy as np

S = 2048          # sequence length
D = 512           # model dim
EL = 256          # local (per-core) slice of model dim = 4 heads * 64
H = 4             # local heads
DH = 64           # head dim
P = 128           # partitions
NKC = D // P      # k chunks for projections (4)
NST = S // P      # s tiles of 128 (16)
NQB = S // 512    # q blocks of 512 (4)

_CACHE = {}


def _build_nc(pt_bufs=6, sc_bufs=2, av_bufs=2, yp_bufs=2):
    import concourse.bacc as bacc
    import concourse.mybir as mybir
    import concourse.tile as tile

    F32 = mybir.dt.float32
    F32R = mybir.dt.float32r
    BF16 = mybir.dt.bfloat16
    EXP = mybir.ActivationFunctionType.Exp

    nc = bacc.Bacc()

    XT = nc.declare_dram_parameter("XT", [D, S], F32R, isOutput=False)
    WQT = nc.declare_dram_parameter("WQT", [D, EL], F32R, isOutput=False)
    WKT = nc.declare_dram_parameter("WKT", [D, EL], F32R, isOutput=False)
    WVT = nc.declare_dram_parameter("WVT", [D, EL], F32R, isOutput=False)
    WOT = nc.declare_dram_parameter("WOT", [EL, D], F32R, isOutput=False)
    BQ = nc.declare_dram_parameter("BQ", [EL], F32, isOutput=False)
    BK = nc.declare_dram_parameter("BK", [EL], F32, isOutput=False)
    BVB = nc.declare_dram_parameter("BVB", [P, EL], F32, isOutput=False)
    Y = nc.declare_dram_parameter("Y", [S, D], F32, isOutput=True)

    with tile.TileContext(nc) as tc:
        with (
            tc.tile_pool(name="persist", bufs=1) as pp,
            tc.tile_pool(name="pt", bufs=pt_bufs) as ptp,
            tc.tile_pool(name="rc", bufs=2) as rcp,
            tc.tile_pool(name="rb", bufs=2) as rbp,
            tc.tile_pool(name="ysb", bufs=3) as ysbp,
            tc.tile_pool(name="sc", bufs=sc_bufs, space="PSUM") as scp,
            tc.tile_pool(name="av", bufs=av_bufs, space="PSUM") as avp,
            tc.tile_pool(name="yp", bufs=yp_bufs, space="PSUM") as ypp,
        ):
            # ---- persistent SBUF tiles ----
            xt = pp.tile([P, NKC, S], F32R)          # X^T, d on partitions
            wqt = pp.tile([P, NKC, EL], F32R)
            wkt = pp.tile([P, NKC, EL], F32R)
            wvt = pp.tile([P, NKC, EL], F32R)
            wot = pp.tile([DH, H, D], F32R)          # d_local on partitions 0-63
            bq = pp.tile([P, 2], F32)
            bk = pp.tile([P, 2], F32)
            bvb = pp.tile([P, EL], F32)
            qt = pp.tile([P, 2, S], F32R)            # Q^T (e-chunk on outer)
            kt = pp.tile([P, 2, S], F32R)
            vv = pp.tile([P, NST, H, DH + 1], F32R)  # V + ones col per head
            at = pp.tile([DH, H, S], F32R)           # normalized attn^T

            # ---- loads: small weight/bias tensors first, X^T in per-512-seq
            # chunks so the first projections + attention start after ~1MB ----
            nc.sync.dma_start(wkt[:], WKT.rearrange("(o p) e -> p o e", p=P))
            nc.sync.dma_start(wqt[:], WQT.rearrange("(o p) e -> p o e", p=P))
            nc.sync.dma_start(bq[:], BQ.rearrange("(o p) -> p o", p=P))
            nc.sync.dma_start(bk[:], BK.rearrange("(o p) -> p o", p=P))
            xt_src = XT.rearrange("(o p) s -> p o s", p=P)
            for sb in range(4):
                nc.sync.dma_start(
                    xt[:, :, sb * 512 : (sb + 1) * 512],
                    xt_src[:, :, sb * 512 : (sb + 1) * 512],
                )
            nc.sync.dma_start(wvt[:], WVT.rearrange("(o p) e -> p o e", p=P))
            nc.sync.dma_start(bvb[:], BVB[:])
            nc.sync.dma_start(wot[:], WOT.rearrange("(h p) e -> p h e", p=DH))

            # fill V tile with 1.0 (broadcast copy; memset can't write f32r);
            # V-proj copies overwrite cols 0:DH of each head, leaving col DH
            # as the ones column that produces the softmax denominator
            ones = pp.tile([P, 1, 1, 1], F32)
            nc.vector.memset(ones[:], 1.0)
            nc.vector.tensor_copy(vv[:], ones[:].to_broadcast([P, NST, H, DH + 1]))
            # ones row at partition 64, lhsT of the K=1 broadcast matmul
            ones2 = pp.tile([DH + 1, DH], F32R)
            nc.vector.tensor_copy(
                ones2[:], ones[0 : DH + 1, 0, 0, :].to_broadcast([DH + 1, DH])
            )

            # ---- emission helpers ----
            def proj_qk(dst, w, b, e, blk):
                """Project one 1024-wide seq block of Q^T or K^T (e-chunk e)."""
                ps = scp.tile([P, 1024], F32, tag="sc")
                for sb in range(2):
                    s0 = (blk * 2 + sb) * 512
                    for k in range(NKC):
                        nc.tensor.matmul(
                            ps[:, sb * 512 : (sb + 1) * 512],
                            w[:, k, e * P : (e + 1) * P],
                            xt[:, k, s0 : s0 + 512],
                            start=(k == 0),
                            stop=(k == NKC - 1),
                            skip_group_check=True,
                        )
                nc.vector.tensor_scalar_add(
                    dst[:, e, blk * 1024 : (blk + 1) * 1024],
                    ps[:],
                    b[:, e : e + 1],
                )

            def proj_v(st):
                ps = ypp.tile([P, 512], F32, tag="yp")
                for k in range(NKC):
                    nc.tensor.matmul(
                        ps[:, :EL],
                        xt[:, k, st * P : (st + 1) * P],
                        wvt[:, k, :],
                        start=(k == 0),
                        stop=(k == NKC - 1),
                        skip_group_check=True,
                    )
                nc.vector.tensor_add(
                    vv[:, st, :, 0:DH],
                    ps[:, :EL].rearrange("p (h d) -> p h d", h=H),
                    bvb[:].rearrange("p (h d) -> p h d", h=H),
                )

            def strips(qb, p, pre_hook=None):
                """Score + exp + attnV accumulation for one (q-block, pair)."""
                q0 = qb * 512
                avA = avp.tile([DH + 1, 512], F32, tag="av")
                avB = avp.tile([DH + 1, 512], F32, tag="av")
                for ks in range(NST):
                    if pre_hook is not None:
                        pre_hook(ks)
                    sc = scp.tile([P, 1024], F32, tag="sc")
                    k0 = ks * P
                    nc.tensor.matmul(
                        sc[:, 0:512],
                        kt[0:DH, p, k0 : k0 + P],
                        qt[0:DH, p, q0 : q0 + 512],
                    )
                    nc.tensor.matmul(
                        sc[:, 512:1024],
                        kt[DH:P, p, k0 : k0 + P],
                        qt[DH:P, p, q0 : q0 + 512],
                    )
                    pt = ptp.tile([P, 1024], F32R, tag="pt")
                    nc.scalar.activation(pt[:], sc[:], EXP)
                    for hp, av in ((0, avA), (1, avB)):
                        h = 2 * p + hp
                        nc.tensor.matmul(
                            av[:],
                            vv[:, ks, h, :],
                            pt[:, hp * 512 : (hp + 1) * 512],
                            start=(ks == 0),
                            stop=(ks == NST - 1),
                            skip_group_check=True,
                        )
                return avA, avB

            def trans(qb, p, avA, avB):
                """Normalize: denom -> bcast matmul -> fast recip -> mult."""
                q0 = qb * 512
                for hp, av in ((0, avA), (1, avB)):
                    h = 2 * p + hp
                    # denominator row -> SBUF (f32r), broadcast across 64
                    # partitions with a K=1 matmul, fast reciprocal, multiply
                    dn = rcp.tile([DH + 1, 512], F32R, tag="rc")
                    nc.vector.tensor_copy(dn[DH : DH + 1, :], av[DH : DH + 1, :])
                    rb_ps = ypp.tile([DH, 512], F32, tag="yp")
                    nc.tensor.matmul(
                        rb_ps[:], ones2[DH : DH + 1, :], dn[DH : DH + 1, :]
                    )
                    rb = rbp.tile([DH, 512], F32, tag="rb")
                    nc.vector.reciprocal_approx_fast(rb[:], rb_ps[:])
                    nc.vector.tensor_mul(
                        at[:, h, q0 : q0 + 512], av[0:DH, :], rb[:]
                    )

            def ytile(st):
                yp = ypp.tile([P, 512], F32, tag="yp")
                for h in range(H):
                    nc.tensor.matmul(
                        yp[:],
                        at[:, h, st * P : (st + 1) * P],
                        wot[:, h, :],
                        start=(h == 0),
                        stop=(h == H - 1),
                        skip_group_check=True,
                    )
                ysb = ysbp.tile([P, 512], F32, tag="ysb")
                nc.vector.tensor_copy(ysb[:], yp[:])
                nc.sync.dma_start(Y[st * P : (st + 1) * P, :], ysb[:])

            # ---- emission order (EMIT_MODE): 1 = interleaved (default,
            # starts attention ASAP), 0 = sequential (debug fallback) ----
            import os as _os

            if _os.environ.get("EMIT_MODE", "1") == "1":
                # kt[e0] fully + qt[e0] blk0 upfront; V projection rides
                # inside the first strips loop; kt[e1] between the two pairs.
                proj_qk(kt, wkt, bk, 0, 0)
                proj_qk(kt, wkt, bk, 0, 1)
                proj_qk(qt, wqt, bq, 0, 0)

                def hook00(ks):
                    # V tiles for this strip + pair-1's K/Q spread through
                    # the tail of the loop so PE slack absorbs them
                    proj_v(ks)
                    if ks == 8:
                        proj_qk(kt, wkt, bk, 1, 0)
                    elif ks == 11:
                        proj_qk(kt, wkt, bk, 1, 1)
                    elif ks == 14:
                        proj_qk(qt, wqt, bq, 1, 0)

                a0, b0 = strips(0, 0, pre_hook=hook00)
                trans(0, 0, a0, b0)

                def hook01(ks):
                    if ks == 4:
                        proj_qk(qt, wqt, bq, 0, 1)
                    elif ks == 10:
                        proj_qk(qt, wqt, bq, 1, 1)

                a0, b0 = strips(0, 1, pre_hook=hook01)
                trans(0, 1, a0, b0)

                for qb in range(1, NQB):
                    # spread the previous q-block's y-projection through the
                    # strip loop so its matmuls fill PE slack between exps
                    def yhook(ks, _q=qb):
                        if ks % 4 == 2:
                            ytile((_q - 1) * 4 + ks // 4)

                    a0, b0 = strips(qb, 0, pre_hook=yhook)
                    trans(qb, 0, a0, b0)
                    a0, b0 = strips(qb, 1)
                    trans(qb, 1, a0, b0)
                for st in range((NQB - 1) * 4, NQB * 4):
                    ytile(st)
            else:
                for e in range(2):
                    for blk in range(2):
                        proj_qk(qt, wqt, bq, e, blk)
                        proj_qk(kt, wkt, bk, e, blk)
                for st in range(NST):
                    proj_v(st)
                for qb in range(NQB):
                    for p in range(2):
                        a0, b0 = strips(qb, p)
                        trans(qb, p, a0, b0)
                    for st in range(qb * 4, (qb + 1) * 4):
                        ytile(st)

    nc.finalize()
    return nc


def _get_nc():
    if "nc" not in _CACHE:
        _CACHE["nc"] = _build_nc()
    return _CACHE["nc"]


def _prep_inputs(X, Wq, bq, Wk, bk, Wv, bv, Wo, bo):
    f = lambda a: np.ascontiguousarray(np.asarray(a), dtype=np.float32)
    X, Wq, bq, Wk, bk, Wv, bv, Wo, bo = map(f, (X, Wq, bq, Wk, bk, Wv, bv, Wo, bo))
    B = X.shape[0]
    scale = np.float32(1.0 / np.sqrt(DH))
    XT = [np.ascontiguousarray(X[b].T) for b in range(B)]
    in_maps = []
    for c in range(2 * B):
        b, hg = divmod(c, 2)
        sl = slice(hg * EL, (hg + 1) * EL)
        in_maps.append(
            {
                "XT": XT[b],
                "WQT": np.ascontiguousarray((Wq[sl] * scale).T),
                "WKT": np.ascontiguousarray(Wk[sl].T),
                "WVT": np.ascontiguousarray(Wv[sl].T),
                "WOT": np.ascontiguousarray(Wo[:, sl].T),
                "BQ": np.ascontiguousarray(bq[sl] * scale),
                "BK": np.ascontiguousarray(bk[sl]),
                "BVB": np.ascontiguousarray(np.tile(bv[sl], (P, 1))),
            }
        )
    return in_maps, bo, B


def run(inputs, trace=False, trace_cores=None):
    """Run the kernel; returns (Y_full, exec_time_ns or None)."""
    from concourse.bass_utils import run_bass_kernel_spmd

    in_maps, bo, B = _prep_inputs(**inputs)
    nc = _get_nc()
    kw = {}
    if trace:
        kw = dict(trace=True, trace_cores=trace_cores or list(range(2 * B)))
    res = run_bass_kernel_spmd(nc, in_maps, list(range(2 * B)), **kw)
    Y = np.stack(
        [
            res.results[2 * b]["Y"] + res.results[2 * b + 1]["Y"] + bo
            for b in range(B)
        ]
    )
    return Y, getattr(res, "exec_time_ns", None)


def kernel(X, Wq, bq, Wk, bk, Wv, bv, Wo, bo):
    Y, _ = run(
        dict(X=X, Wq=Wq, bq=bq, Wk=Wk, bk=bk, Wv=Wv, bv=bv, Wo=Wo, bo=bo)
    )
    return Y
